# revision 1
# baseline (speedup 1.0000x reference)
"""Trainium2 Bass kernel for nn_AttentionMax (batched dot-product argmax one-hot).

corr[b, s] = <feat_query[b], feat_sub[b, s]>   (bz=4096, n_support=256, d=128)
out[b, s, 0] = one_hot(argmax_s corr[b])

Sharding: pure data parallel over the batch dim across 8 NeuronCores
(512 batches per core = blocks of 128; partition dim = batch).

Active strategy (VARIANT="v4"): feat_sub is transposed on the host to
[b, d, s] so each [P, DH, NS] slot DMAs contiguously.  Per slot, the
multiplies prod[d] = sub[d] * q[d] are split between VectorE (one big
tensor_tensor with q 0-stride-broadcast along s) and ScalarE (per-d
activation Identity with per-partition scale) — both engines run at
full tilt in parallel.  The reduction over d is a log2 in-place fold of
tensor_tensor adds (all ops innermost-contiguous; TENSOR_REDUCE over a
strided view measured 1.6x slower, and fp32 PE matmul is a non-starter
since it lowers to 2 LDWEIGHTS+MATMUL passes).  The first-argmax
one-hot is exact (ties resolve to the lowest index, matching
jnp.argmax) via reduce_max -> (corr==max)*(iota-1024) -> reduce_min ->
(iota-1024==min), computed entirely on VectorE.

Measured on hardware: ~230.7 us end-to-end per core, exact output
(memory roofline for the 512 MiB feat_sub stream is ~180 us; VectorE +
ScalarE combined fp32 throughput ~1.38 elem/ns is the binding
constraint; DVE 199 us / ACT 194 us busy, both >82%, ~25 us residual
startup ramp + drain-barrier latency).  Config: DH=16 slots, D_ACT
11/12 alternating, DMA split 2 (4 for the first two slots), ScalarE
takes the leading d's of each slot so it starts on the first chunk,
fold stops at a [P, 4, NS] running accumulator, and compute reads the
q/iota DMA tiles directly (Bacc legalizes multi-wait instructions, so
no staging copies are needed).
"""

import sys

if "/opt/trn_rl_repo" not in sys.path:
    sys.path.insert(0, "/opt/trn_rl_repo")

import numpy as np

import concourse.bass as bass
import concourse.mybir as mybir
from concourse import bacc, tile
from concourse.bass_utils import run_bass_kernel_spmd

N_CORES = 8
BZ = 4096
BZL = BZ // N_CORES  # 512 batches per core
NS = 256  # n_support
D = 128
P = 128  # batches per block (partition dim)
NBLK = BZL // P  # 4

# v2 layout params
DH = 16  # d-slice width per DMA slot
NH = D // DH  # slots per block
D_ACT = 11  # per-slot count of d's offloaded to ScalarE (v3/v4)
DMA_SPLIT = 2  # per-slot DMA split (v4): finer chunks -> earlier compute start
ACC_DMA = False  # SWDGE accum DMA for corr2: measured 27 us WORSE (serial RMW latency)

VARIANT = "v4"

F32 = mybir.dt.float32


def _argmax_onehot(nc, c_pool, iota_v, acc, out, b0):
    """Exact first-argmax one-hot from acc [P, NS] -> DMA to out[b0:b0+P]."""
    rmax = c_pool.tile([P, 1], F32)
    nc.vector.reduce_max(out=rmax[:], in_=acc[:], axis=mybir.AxisListType.X)
    masked = c_pool.tile([P, NS], F32)
    nc.vector.scalar_tensor_tensor(
        out=masked[:], in0=acc[:], scalar=rmax[:], in1=iota_v[:],
        op0=mybir.AluOpType.is_equal, op1=mybir.AluOpType.mult,
    )
    rmin = c_pool.tile([P, 1], F32)
    nc.vector.tensor_reduce(
        out=rmin[:], in_=masked[:], axis=mybir.AxisListType.X,
        op=mybir.AluOpType.min,
    )
    onehot = c_pool.tile([P, NS], F32)
    nc.vector.tensor_scalar(
        out=onehot[:], in0=iota_v[:], scalar1=rmin[:], scalar2=None,
        op0=mybir.AluOpType.is_equal,
    )
    nc.scalar.dma_start(out=out[b0 : b0 + P, :], in_=onehot[:])


def _build_v2():
    nc = bacc.Bacc("TRN2", target_bir_lowering=False, debug=False)
    fq = nc.declare_dram_parameter("feat_query", [BZL, D], F32, isOutput=False)
    fs = nc.declare_dram_parameter("feat_sub", [BZL, NH, NS, DH], F32, isOutput=False)
    iota = nc.declare_dram_parameter("iota", [P, NS], F32, isOutput=False)
    out = nc.declare_dram_parameter("out", [BZL, NS], F32, isOutput=True)

    n_act = D_ACT  # per-slot count of ACT-offloaded d's
    with tile.TileContext(nc) as tc:
        with (
            tc.tile_pool(name="sub", bufs=3) as sub_pool,
            tc.tile_pool(name="qp", bufs=NBLK) as q_pool,
            tc.tile_pool(name="cp", bufs=NBLK) as c_pool,
            tc.tile_pool(name="pa", bufs=2) as pa_pool,
            tc.tile_pool(name="const", bufs=1) as const_pool,
        ):
            iota_d = const_pool.tile([P, NS], F32)
            nc.scalar.dma_start(out=iota_d[:], in_=iota[:, :])
            iota_v = const_pool.tile([P, NS], F32)
            nc.vector.tensor_copy(iota_v[:], iota_d[:])

            for blk in range(NBLK):
                b0 = blk * P
                q_d = q_pool.tile([P, D], F32)
                nc.scalar.dma_start(out=q_d[:], in_=fq[b0 : b0 + P, :])
                q_v = q_pool.tile([P, D], F32)
                nc.vector.tensor_copy(q_v[:], q_d[:])
                # ScalarE also needs q as its scale operand; give it its own
                # copy so ACT ops don't add cross-engine waits against DVE.
                if n_act:
                    q_a = q_pool.tile([P, D], F32)
                    nc.scalar.activation(
                        out=q_a[:], in_=q_d[:],
                        func=mybir.ActivationFunctionType.Identity,
                    )
                    prod_a = pa_pool.tile([P, NS, NH * n_act], F32)
                acc = c_pool.tile([P, NS], F32)

                for h in range(NH):
                    sub_tile = sub_pool.tile([P, NS, DH], F32)
                    nc.sync.dma_start(out=sub_tile[:], in_=fs[b0 : b0 + P, h, :, :])
                    n_dve = DH - n_act
                    for dd in range(n_dve):
                        d = h * DH + dd
                        if d == 0:
                            nc.vector.tensor_scalar(
                                out=acc[:], in0=sub_tile[:, :, 0],
                                scalar1=q_v[:, 0:1], scalar2=None,
                                op0=mybir.AluOpType.mult,
                            )
                        else:
                            nc.vector.scalar_tensor_tensor(
                                out=acc[:], in0=sub_tile[:, :, dd],
                                scalar=q_v[:, d : d + 1], in1=acc[:],
                                op0=mybir.AluOpType.mult, op1=mybir.AluOpType.add,
                            )
                    for j in range(n_act):
                        dd = n_dve + j
                        d = h * DH + dd
                        nc.scalar.activation(
                            out=prod_a[:, :, h * n_act + j], in_=sub_tile[:, :, dd],
                            func=mybir.ActivationFunctionType.Identity,
                            scale=q_a[:, d : d + 1],
                        )

                if n_act:
                    psum_a = c_pool.tile([P, NS], F32)
                    nc.vector.reduce_sum(
                        out=psum_a[:], in_=prod_a[:], axis=mybir.AxisListType.X
                    )
                    nc.vector.tensor_tensor(
                        out=acc[:], in0=acc[:], in1=psum_a[:], op=mybir.AluOpType.add
                    )

                _argmax_onehot(nc, c_pool, iota_v, acc, out, b0)

    nc.compile()
    return nc


def _build_v3():
    """Layout [b, d, s]: slots [P, DH, NS] (contiguous per partition).

    Per slot of DH d-values: DVE multiplies the first DH-D_ACT d's in one
    big tensor_tensor (q broadcast along s), ScalarE multiplies the other
    D_ACT d's (contiguous activations with per-partition scale) into the
    same prod tile.  DVE then reduce_sums the slot over d via an s-major
    strided view and accumulates partial correlations.
    """
    nc = bacc.Bacc("TRN2", target_bir_lowering=False, debug=False)
    fq = nc.declare_dram_parameter("feat_query", [BZL, D], F32, isOutput=False)
    fs = nc.declare_dram_parameter("feat_sub", [BZL, D, NS], F32, isOutput=False)
    iota = nc.declare_dram_parameter("iota", [P, NS], F32, isOutput=False)
    out = nc.declare_dram_parameter("out", [BZL, NS], F32, isOutput=True)

    n_act = D_ACT
    n_dve = DH - n_act
    with tile.TileContext(nc) as tc:
        with (
            tc.tile_pool(name="sub", bufs=3) as sub_pool,
            tc.tile_pool(name="prod", bufs=2) as prod_pool,
            tc.tile_pool(name="qp", bufs=NBLK) as q_pool,
            tc.tile_pool(name="cp", bufs=NBLK) as c_pool,
            tc.tile_pool(name="const", bufs=1) as const_pool,
        ):
            iota_d = const_pool.tile([P, NS], F32)
            nc.scalar.dma_start(out=iota_d[:], in_=iota[:, :])
            iota_v = const_pool.tile([P, NS], F32)
            nc.vector.tensor_copy(iota_v[:], iota_d[:])

            for blk in range(NBLK):
                b0 = blk * P
                q_d = q_pool.tile([P, D], F32)
                nc.scalar.dma_start(out=q_d[:], in_=fq[b0 : b0 + P, :])
                q_v = q_pool.tile([P, D], F32)
                nc.vector.tensor_copy(q_v[:], q_d[:])
                q_a = q_pool.tile([P, D], F32)
                nc.scalar.activation(
                    out=q_a[:], in_=q_d[:],
                    func=mybir.ActivationFunctionType.Identity,
                )
                corr = c_pool.tile([P, NS], F32)

                for h in range(NH):
                    d0 = h * DH
                    sub_tile = sub_pool.tile([P, DH, NS], F32)
                    nc.sync.dma_start(out=sub_tile[:], in_=fs[b0 : b0 + P, d0 : d0 + DH, :])
                    prod = prod_pool.tile([P, DH, NS], F32)
                    if n_dve:
                        q_b = (
                            q_v[:, d0 : d0 + n_dve]
                            .unsqueeze(2)
                            .broadcast_to([P, n_dve, NS])
                        )
                        nc.vector.tensor_tensor(
                            out=prod[:, 0:n_dve, :], in0=sub_tile[:, 0:n_dve, :],
                            in1=q_b, op=mybir.AluOpType.mult,
                        )
                    for j in range(n_act):
                        dd = n_dve + j
                        nc.scalar.activation(
                            out=prod[:, dd, :], in_=sub_tile[:, dd, :],
                            func=mybir.ActivationFunctionType.Identity,
                            scale=q_a[:, d0 + dd : d0 + dd + 1],
                        )
                    # reduce over d via s-major strided view
                    psum_h = c_pool.tile([P, NS], F32)
                    nc.vector.reduce_sum(
                        out=psum_h[:],
                        in_=prod[:].rearrange("p d s -> p s d"),
                        axis=mybir.AxisListType.X,
                    )
                    if h == 0:
                        first = psum_h
                    else:
                        nc.vector.tensor_tensor(
                            out=corr[:] if h == NH - 1 else first[:],
                            in0=first[:], in1=psum_h[:], op=mybir.AluOpType.add,
                        )

                _argmax_onehot(nc, c_pool, iota_v, corr, out, b0)

    nc.compile()
    return nc


def _build_v4():
    """Layout [b, d, s] with TT-add fold reduction (all ops inner-contiguous).

    Per slot of DH=32 d-values: DVE multiplies the first DH-D_ACT d's in one
    tensor_tensor (q broadcast along s), ScalarE multiplies the other D_ACT
    d's (contiguous in/out, per-partition scale).  The d-reduction is a
    log2 fold of in-place tensor_tensor adds on [P, k, NS] slices -- every
    op reads/writes s-contiguous memory (no strided TENSOR_REDUCE).
    """
    nc = bacc.Bacc("TRN2", target_bir_lowering=False, debug=False)
    fq = nc.declare_dram_parameter("feat_query", [BZL, D], F32, isOutput=False)
    fs = nc.declare_dram_parameter("feat_sub", [BZL, D, NS], F32, isOutput=False)
    iota = nc.declare_dram_parameter("iota", [P, NS], F32, isOutput=False)
    out = nc.declare_dram_parameter("out", [BZL, NS], F32, isOutput=True)

    with tile.TileContext(nc) as tc:
        with (
            tc.tile_pool(name="sub", bufs=4) as sub_pool,
            tc.tile_pool(name="prod", bufs=5) as prod_pool,
            tc.tile_pool(name="qp", bufs=NBLK) as q_pool,
            tc.tile_pool(name="cp", bufs=NBLK) as c_pool,
            tc.tile_pool(name="const", bufs=1) as const_pool,
        ):
            iota_v = const_pool.tile([P, NS], F32)
            nc.scalar.dma_start(out=iota_v[:], in_=iota[:, :])

            for blk in range(NBLK):
                b0 = blk * P
                q_v = q_pool.tile([P, D], F32)
                nc.scalar.dma_start(out=q_v[:], in_=fq[b0 : b0 + P, :])
                q_a = q_pool.tile([P, D], F32)
                nc.scalar.activation(
                    out=q_a[:], in_=q_v[:],
                    func=mybir.ActivationFunctionType.Identity,
                )
                corr4 = c_pool.tile([P, 4, NS], F32)

                for h in range(NH):
                    d0 = h * DH
                    # alternate ScalarE share to balance engine busy-time;
                    # ACT-heavier at the tail of the last block so the final
                    # DVE-only fold+argmax stretch is shorter
                    n_act = D_ACT + ((blk * NH + h) % 2)
                    if blk == NBLK - 1 and h >= NH - 2:
                        n_act += 2
                    n_dve = DH - n_act
                    sub_tile = sub_pool.tile([P, DH, NS], F32)
                    # finer chunks for the first two slots: compute starts
                    # during the DMA ramp instead of after the first 1 MB
                    nsplit = 4 if (blk == 0 and h <= 1) else DMA_SPLIT
                    dstep = DH // nsplit
                    for c in range(nsplit):
                        nc.sync.dma_start(
                            out=sub_tile[:, c * dstep : (c + 1) * dstep, :],
                            in_=fs[b0 : b0 + P, d0 + c * dstep : d0 + (c + 1) * dstep, :],
                        )
                    prod = prod_pool.tile([P, DH, NS], F32)
                    # ScalarE takes the LEADING d's: they arrive in the first
                    # DMA chunk, so ACT starts as early as possible; VectorE
                    # multiplies the trailing d's in one big tensor_tensor.
                    # Exception: the very first slot flips the assignment --
                    # at kernel start the DMA ramp is slow and VectorE (the
                    # critical engine) would otherwise idle ~10 us waiting
                    # for the last chunks.
                    dve_first = False
                    dlo = 0 if dve_first else n_act  # first DVE row
                    alo = n_dve if dve_first else 0  # first ACT row
                    for j in range(n_act):
                        nc.scalar.activation(
                            out=prod[:, alo + j, :], in_=sub_tile[:, alo + j, :],
                            func=mybir.ActivationFunctionType.Identity,
                            scale=q_a[:, d0 + alo + j : d0 + alo + j + 1],
                        )
                    if n_dve:
                        q_b = (
                            q_v[:, d0 + dlo : d0 + dlo + n_dve]
                            .unsqueeze(2)
                            .broadcast_to([P, n_dve, NS])
                        )
                        nc.vector.tensor_tensor(
                            out=prod[:, dlo : dlo + n_dve, :],
                            in0=sub_tile[:, dlo : dlo + n_dve, :],
                            in1=q_b, op=mybir.AluOpType.mult,
                        )
                    # in-place halving fold over d: 16 -> 8 -> 4, then
                    # accumulate the [P, 4, NS] remainder (one fewer small
                    # op per slot than folding all the way to 2 rows)
                    k = DH // 2
                    while k >= 4:
                        nc.vector.tensor_tensor(
                            out=prod[:, 0:k, :], in0=prod[:, 0:k, :],
                            in1=prod[:, k : 2 * k, :], op=mybir.AluOpType.add,
                        )
                        k //= 2
                    if h == 0:
                        nc.vector.tensor_copy(corr4[:], prod[:, 0:4, :])
                    else:
                        nc.vector.tensor_tensor(
                            out=corr4[:], in0=corr4[:], in1=prod[:, 0:4, :],
                            op=mybir.AluOpType.add,
                        )

                # fold corr4 to a single [P, NS] row in place
                nc.vector.tensor_tensor(
                    out=corr4[:, 0:2, :], in0=corr4[:, 0:2, :],
                    in1=corr4[:, 2:4, :], op=mybir.AluOpType.add,
                )
                nc.vector.tensor_tensor(
                    out=corr4[:, 0, :], in0=corr4[:, 0, :], in1=corr4[:, 1, :],
                    op=mybir.AluOpType.add,
                )
                _argmax_onehot(nc, c_pool, iota_v, corr4[:, 0, :], out, b0)

    nc.compile()
    return nc


SC = 64  # v1 s-chunk


def _build_v1():
    nc = bacc.Bacc("TRN2", target_bir_lowering=False, debug=False)
    fq = nc.declare_dram_parameter("feat_query", [BZL, D], F32, isOutput=False)
    fs = nc.declare_dram_parameter("feat_sub", [BZL, NS, D], F32, isOutput=False)
    iota = nc.declare_dram_parameter("iota", [P, NS], F32, isOutput=False)
    out = nc.declare_dram_parameter("out", [BZL, NS], F32, isOutput=True)

    with tile.TileContext(nc) as tc:
        with (
            tc.tile_pool(name="sub", bufs=3) as sub_pool,
            tc.tile_pool(name="prod", bufs=2) as prod_pool,
            tc.tile_pool(name="qp", bufs=NBLK) as q_pool,
            tc.tile_pool(name="cp", bufs=NBLK) as c_pool,
            tc.tile_pool(name="const", bufs=1) as const_pool,
        ):
            iota_d = const_pool.tile([P, NS], F32)
            nc.scalar.dma_start(out=iota_d[:], in_=iota[:, :])
            iota_v = const_pool.tile([P, NS], F32)
            nc.vector.tensor_copy(iota_v[:], iota_d[:])

            for blk in range(NBLK):
                b0 = blk * P
                q_d = q_pool.tile([P, D], F32)
                nc.scalar.dma_start(out=q_d[:], in_=fq[b0 : b0 + P, :])
                q_v = q_pool.tile([P, D], F32)
                nc.vector.tensor_copy(q_v[:], q_d[:])
                corr = c_pool.tile([P, NS], F32)

                for ci in range(NS // SC):
                    sub_tile = sub_pool.tile([P, SC, D], F32)
                    nc.sync.dma_start(
                        out=sub_tile[:],
                        in_=fs[b0 : b0 + P, ci * SC : (ci + 1) * SC, :],
                    )
                    prod = prod_pool.tile([P, SC, D], F32)
                    q_b = q_v[:, :].unsqueeze(1).broadcast_to([P, SC, D])
                    nc.vector.tensor_tensor(
                        out=prod[:], in0=sub_tile[:], in1=q_b, op=mybir.AluOpType.mult
                    )
                    nc.vector.reduce_sum(
                        out=corr[:, ci * SC : (ci + 1) * SC],
                        in_=prod[:],
                        axis=mybir.AxisListType.X,
                    )

                _argmax_onehot(nc, c_pool, iota_v, corr, out, b0)

    nc.compile()
    return nc


_CACHE = {}


def _get_nc():
    key = f"{VARIANT}-{DH}-{D_ACT}-{ACC_DMA}"
    if key not in _CACHE:
        builders = {"v1": _build_v1, "v2": _build_v2, "v3": _build_v3, "v4": _build_v4}
        _CACHE[key] = builders[VARIANT]()
    return _CACHE[key]


def _in_maps(feat_query, feat_sub):
    feat_query = np.ascontiguousarray(np.asarray(feat_query), dtype=np.float32)
    feat_sub = np.asarray(feat_sub)
    assert feat_query.shape == (BZ, D), feat_query.shape
    assert feat_sub.shape == (BZ, NS, D), feat_sub.shape
    if VARIANT == "v2":
        # host-side reorder: [BZ, NS, D] -> [BZ, NH, NS, DH] (d-slices contiguous)
        feat_sub = np.ascontiguousarray(
            feat_sub.reshape(BZ, NS, NH, DH).transpose(0, 2, 1, 3), dtype=np.float32
        )
    elif VARIANT in ("v3", "v4"):
        # host-side transpose: [BZ, NS, D] -> [BZ, D, NS]
        feat_sub = np.ascontiguousarray(
            feat_sub.transpose(0, 2, 1), dtype=np.float32
        )
    else:
        feat_sub = np.ascontiguousarray(feat_sub, dtype=np.float32)
    iota_np = np.tile(np.arange(NS, dtype=np.float32) - 1024.0, (P, 1))
    maps = []
    for i in range(N_CORES):
        sl = slice(i * BZL, (i + 1) * BZL)
        maps.append(
            {"feat_query": feat_query[sl], "feat_sub": feat_sub[sl], "iota": iota_np}
        )
    return maps


def _assemble(results):
    outs = [results[i]["out"] for i in range(N_CORES)]
    return np.concatenate(outs, axis=0).reshape(BZ, NS, 1).astype(np.float32)


def run(feat_query, feat_sub, trace=False):
    """Run on 8 NeuronCores; returns (output, BassKernelResults)."""
    nc = _get_nc()
    res = run_bass_kernel_spmd(
        nc, _in_maps(feat_query, feat_sub), list(range(N_CORES)), trace=trace
    )
    return _assemble(res.results), res


def kernel(feat_query, feat_sub):
    out, _ = run(feat_query, feat_sub, trace=False)
    return out



# revision 3
# speedup vs baseline: 1.0105x; 1.0105x over previous
"""Trainium2 Bass kernel for nn_AttentionMax (batched dot-product argmax one-hot).

corr[b, s] = <feat_query[b], feat_sub[b, s]>   (bz=4096, n_support=256, d=128)
out[b, s, 0] = one_hot(argmax_s corr[b])

Sharding: pure data parallel over the batch dim across 8 NeuronCores
(512 batches per core = blocks of 128; partition dim = batch).

Active strategy (VARIANT="v4"): feat_sub is transposed on the host to
[b, d, s] so each [P, DH, NS] slot DMAs contiguously.  Per slot, the
multiplies prod[d] = sub[d] * q[d] are split between VectorE (one big
tensor_tensor with q 0-stride-broadcast along s) and ScalarE (per-d
activation Identity with per-partition scale) — both engines run at
full tilt in parallel.  The reduction over d is a log2 in-place fold of
tensor_tensor adds (all ops innermost-contiguous; TENSOR_REDUCE over a
strided view measured 1.6x slower, and fp32 PE matmul is a non-starter
since it lowers to 2 LDWEIGHTS+MATMUL passes).  The first-argmax
one-hot is exact (ties resolve to the lowest index, matching
jnp.argmax) via reduce_max -> (corr==max)*(iota-1024) -> reduce_min ->
(iota-1024==min), computed entirely on VectorE.

Measured on hardware: ~230.7 us end-to-end per core, exact output
(memory roofline for the 512 MiB feat_sub stream is ~180 us; VectorE +
ScalarE combined fp32 throughput ~1.38 elem/ns is the binding
constraint; DVE 199 us / ACT 194 us busy, both >82%, ~25 us residual
startup ramp + drain-barrier latency).  Config: DH=16 slots, D_ACT
11/12 alternating, DMA split 2 (4 for the first two slots), ScalarE
takes the leading d's of each slot so it starts on the first chunk,
fold stops at a [P, 4, NS] running accumulator, and compute reads the
q/iota DMA tiles directly (Bacc legalizes multi-wait instructions, so
no staging copies are needed).
"""

import sys

if "/opt/trn_rl_repo" not in sys.path:
    sys.path.insert(0, "/opt/trn_rl_repo")

import numpy as np

import concourse.bass as bass
import concourse.mybir as mybir
from concourse import bacc, tile
from concourse.bass_utils import run_bass_kernel_spmd

N_CORES = 8
BZ = 4096
BZL = BZ // N_CORES  # 512 batches per core
NS = 256  # n_support
D = 128
P = 128  # batches per block (partition dim)
NBLK = BZL // P  # 4

# v2 layout params
DH = 16  # d-slice width per DMA slot
NH = D // DH  # slots per block
D_ACT = 11  # per-slot count of d's offloaded to ScalarE (v3/v4)
N_C = 5  # v6: ACT->PE rows per slot
N_B = 2  # v6: ACT->DVE-fold rows per slot
DMA_SPLIT = 2  # per-slot DMA split (v4): finer chunks -> earlier compute start
ACC_DMA = False  # SWDGE accum DMA for corr2: measured 27 us WORSE (serial RMW latency)

VARIANT = "v6"

F32 = mybir.dt.float32


def _argmax_onehot(nc, c_pool, iota_v, acc, out, b0):
    """Exact first-argmax one-hot from acc [P, NS] -> DMA to out[b0:b0+P]."""
    rmax = c_pool.tile([P, 1], F32)
    nc.vector.reduce_max(out=rmax[:], in_=acc[:], axis=mybir.AxisListType.X)
    masked = c_pool.tile([P, NS], F32)
    nc.vector.scalar_tensor_tensor(
        out=masked[:], in0=acc[:], scalar=rmax[:], in1=iota_v[:],
        op0=mybir.AluOpType.is_equal, op1=mybir.AluOpType.mult,
    )
    rmin = c_pool.tile([P, 1], F32)
    nc.vector.tensor_reduce(
        out=rmin[:], in_=masked[:], axis=mybir.AxisListType.X,
        op=mybir.AluOpType.min,
    )
    onehot = c_pool.tile([P, NS], F32)
    nc.vector.tensor_scalar(
        out=onehot[:], in0=iota_v[:], scalar1=rmin[:], scalar2=None,
        op0=mybir.AluOpType.is_equal,
    )
    nc.scalar.dma_start(out=out[b0 : b0 + P, :], in_=onehot[:])


def _build_v2():
    nc = bacc.Bacc("TRN2", target_bir_lowering=False, debug=False)
    fq = nc.declare_dram_parameter("feat_query", [BZL, D], F32, isOutput=False)
    fs = nc.declare_dram_parameter("feat_sub", [BZL, NH, NS, DH], F32, isOutput=False)
    iota = nc.declare_dram_parameter("iota", [P, NS], F32, isOutput=False)
    out = nc.declare_dram_parameter("out", [BZL, NS], F32, isOutput=True)

    n_act = D_ACT  # per-slot count of ACT-offloaded d's
    with tile.TileContext(nc) as tc:
        with (
            tc.tile_pool(name="sub", bufs=3) as sub_pool,
            tc.tile_pool(name="qp", bufs=NBLK) as q_pool,
            tc.tile_pool(name="cp", bufs=NBLK) as c_pool,
            tc.tile_pool(name="pa", bufs=2) as pa_pool,
            tc.tile_pool(name="const", bufs=1) as const_pool,
        ):
            iota_d = const_pool.tile([P, NS], F32)
            nc.scalar.dma_start(out=iota_d[:], in_=iota[:, :])
            iota_v = const_pool.tile([P, NS], F32)
            nc.vector.tensor_copy(iota_v[:], iota_d[:])

            for blk in range(NBLK):
                b0 = blk * P
                q_d = q_pool.tile([P, D], F32)
                nc.scalar.dma_start(out=q_d[:], in_=fq[b0 : b0 + P, :])
                q_v = q_pool.tile([P, D], F32)
                nc.vector.tensor_copy(q_v[:], q_d[:])
                # ScalarE also needs q as its scale operand; give it its own
                # copy so ACT ops don't add cross-engine waits against DVE.
                if n_act:
                    q_a = q_pool.tile([P, D], F32)
                    nc.scalar.activation(
                        out=q_a[:], in_=q_d[:],
                        func=mybir.ActivationFunctionType.Identity,
                    )
                    prod_a = pa_pool.tile([P, NS, NH * n_act], F32)
                acc = c_pool.tile([P, NS], F32)

                for h in range(NH):
                    sub_tile = sub_pool.tile([P, NS, DH], F32)
                    nc.sync.dma_start(out=sub_tile[:], in_=fs[b0 : b0 + P, h, :, :])
                    n_dve = DH - n_act
                    for dd in range(n_dve):
                        d = h * DH + dd
                        if d == 0:
                            nc.vector.tensor_scalar(
                                out=acc[:], in0=sub_tile[:, :, 0],
                                scalar1=q_v[:, 0:1], scalar2=None,
                                op0=mybir.AluOpType.mult,
                            )
                        else:
                            nc.vector.scalar_tensor_tensor(
                                out=acc[:], in0=sub_tile[:, :, dd],
                                scalar=q_v[:, d : d + 1], in1=acc[:],
                                op0=mybir.AluOpType.mult, op1=mybir.AluOpType.add,
                            )
                    for j in range(n_act):
                        dd = n_dve + j
                        d = h * DH + dd
                        nc.scalar.activation(
                            out=prod_a[:, :, h * n_act + j], in_=sub_tile[:, :, dd],
                            func=mybir.ActivationFunctionType.Identity,
                            scale=q_a[:, d : d + 1],
                        )

                if n_act:
                    psum_a = c_pool.tile([P, NS], F32)
                    nc.vector.reduce_sum(
                        out=psum_a[:], in_=prod_a[:], axis=mybir.AxisListType.X
                    )
                    nc.vector.tensor_tensor(
                        out=acc[:], in0=acc[:], in1=psum_a[:], op=mybir.AluOpType.add
                    )

                _argmax_onehot(nc, c_pool, iota_v, acc, out, b0)

    nc.compile()
    return nc


def _build_v3():
    """Layout [b, d, s]: slots [P, DH, NS] (contiguous per partition).

    Per slot of DH d-values: DVE multiplies the first DH-D_ACT d's in one
    big tensor_tensor (q broadcast along s), ScalarE multiplies the other
    D_ACT d's (contiguous activations with per-partition scale) into the
    same prod tile.  DVE then reduce_sums the slot over d via an s-major
    strided view and accumulates partial correlations.
    """
    nc = bacc.Bacc("TRN2", target_bir_lowering=False, debug=False)
    fq = nc.declare_dram_parameter("feat_query", [BZL, D], F32, isOutput=False)
    fs = nc.declare_dram_parameter("feat_sub", [BZL, D, NS], F32, isOutput=False)
    iota = nc.declare_dram_parameter("iota", [P, NS], F32, isOutput=False)
    out = nc.declare_dram_parameter("out", [BZL, NS], F32, isOutput=True)

    n_act = D_ACT
    n_dve = DH - n_act
    with tile.TileContext(nc) as tc:
        with (
            tc.tile_pool(name="sub", bufs=3) as sub_pool,
            tc.tile_pool(name="prod", bufs=2) as prod_pool,
            tc.tile_pool(name="qp", bufs=NBLK) as q_pool,
            tc.tile_pool(name="cp", bufs=NBLK) as c_pool,
            tc.tile_pool(name="const", bufs=1) as const_pool,
        ):
            iota_d = const_pool.tile([P, NS], F32)
            nc.scalar.dma_start(out=iota_d[:], in_=iota[:, :])
            iota_v = const_pool.tile([P, NS], F32)
            nc.vector.tensor_copy(iota_v[:], iota_d[:])

            for blk in range(NBLK):
                b0 = blk * P
                q_d = q_pool.tile([P, D], F32)
                nc.scalar.dma_start(out=q_d[:], in_=fq[b0 : b0 + P, :])
                q_v = q_pool.tile([P, D], F32)
                nc.vector.tensor_copy(q_v[:], q_d[:])
                q_a = q_pool.tile([P, D], F32)
                nc.scalar.activation(
                    out=q_a[:], in_=q_d[:],
                    func=mybir.ActivationFunctionType.Identity,
                )
                corr = c_pool.tile([P, NS], F32)

                for h in range(NH):
                    d0 = h * DH
                    sub_tile = sub_pool.tile([P, DH, NS], F32)
                    nc.sync.dma_start(out=sub_tile[:], in_=fs[b0 : b0 + P, d0 : d0 + DH, :])
                    prod = prod_pool.tile([P, DH, NS], F32)
                    if n_dve:
                        q_b = (
                            q_v[:, d0 : d0 + n_dve]
                            .unsqueeze(2)
                            .broadcast_to([P, n_dve, NS])
                        )
                        nc.vector.tensor_tensor(
                            out=prod[:, 0:n_dve, :], in0=sub_tile[:, 0:n_dve, :],
                            in1=q_b, op=mybir.AluOpType.mult,
                        )
                    for j in range(n_act):
                        dd = n_dve + j
                        nc.scalar.activation(
                            out=prod[:, dd, :], in_=sub_tile[:, dd, :],
                            func=mybir.ActivationFunctionType.Identity,
                            scale=q_a[:, d0 + dd : d0 + dd + 1],
                        )
                    # reduce over d via s-major strided view
                    psum_h = c_pool.tile([P, NS], F32)
                    nc.vector.reduce_sum(
                        out=psum_h[:],
                        in_=prod[:].rearrange("p d s -> p s d"),
                        axis=mybir.AxisListType.X,
                    )
                    if h == 0:
                        first = psum_h
                    else:
                        nc.vector.tensor_tensor(
                            out=corr[:] if h == NH - 1 else first[:],
                            in0=first[:], in1=psum_h[:], op=mybir.AluOpType.add,
                        )

                _argmax_onehot(nc, c_pool, iota_v, corr, out, b0)

    nc.compile()
    return nc


def _build_v4():
    """Layout [b, d, s] with TT-add fold reduction (all ops inner-contiguous).

    Per slot of DH=32 d-values: DVE multiplies the first DH-D_ACT d's in one
    tensor_tensor (q broadcast along s), ScalarE multiplies the other D_ACT
    d's (contiguous in/out, per-partition scale).  The d-reduction is a
    log2 fold of in-place tensor_tensor adds on [P, k, NS] slices -- every
    op reads/writes s-contiguous memory (no strided TENSOR_REDUCE).
    """
    nc = bacc.Bacc("TRN2", target_bir_lowering=False, debug=False)
    fq = nc.declare_dram_parameter("feat_query", [BZL, D], F32, isOutput=False)
    fs = nc.declare_dram_parameter("feat_sub", [BZL, D, NS], F32, isOutput=False)
    iota = nc.declare_dram_parameter("iota", [P, NS], F32, isOutput=False)
    out = nc.declare_dram_parameter("out", [BZL, NS], F32, isOutput=True)

    with tile.TileContext(nc) as tc:
        with (
            tc.tile_pool(name="sub", bufs=4) as sub_pool,
            tc.tile_pool(name="prod", bufs=5) as prod_pool,
            tc.tile_pool(name="qp", bufs=NBLK) as q_pool,
            tc.tile_pool(name="cp", bufs=NBLK) as c_pool,
            tc.tile_pool(name="const", bufs=1) as const_pool,
        ):
            iota_v = const_pool.tile([P, NS], F32)
            nc.scalar.dma_start(out=iota_v[:], in_=iota[:, :])

            for blk in range(NBLK):
                b0 = blk * P
                q_v = q_pool.tile([P, D], F32)
                nc.scalar.dma_start(out=q_v[:], in_=fq[b0 : b0 + P, :])
                q_a = q_pool.tile([P, D], F32)
                nc.scalar.activation(
                    out=q_a[:], in_=q_v[:],
                    func=mybir.ActivationFunctionType.Identity,
                )
                corr4 = c_pool.tile([P, 4, NS], F32)

                for h in range(NH):
                    d0 = h * DH
                    # alternate ScalarE share to balance engine busy-time;
                    # ACT-heavier at the tail of the last block so the final
                    # DVE-only fold+argmax stretch is shorter
                    n_act = D_ACT + ((blk * NH + h) % 2)
                    if blk == NBLK - 1 and h >= NH - 2:
                        n_act += 2
                    n_dve = DH - n_act
                    sub_tile = sub_pool.tile([P, DH, NS], F32)
                    # finer chunks for the first two slots: compute starts
                    # during the DMA ramp instead of after the first 1 MB
                    nsplit = 4 if (blk == 0 and h <= 1) else DMA_SPLIT
                    dstep = DH // nsplit
                    for c in range(nsplit):
                        nc.sync.dma_start(
                            out=sub_tile[:, c * dstep : (c + 1) * dstep, :],
                            in_=fs[b0 : b0 + P, d0 + c * dstep : d0 + (c + 1) * dstep, :],
                        )
                    prod = prod_pool.tile([P, DH, NS], F32)
                    # ScalarE takes the LEADING d's: they arrive in the first
                    # DMA chunk, so ACT starts as early as possible; VectorE
                    # multiplies the trailing d's in one big tensor_tensor.
                    # Exception: the very first slot flips the assignment --
                    # at kernel start the DMA ramp is slow and VectorE (the
                    # critical engine) would otherwise idle ~10 us waiting
                    # for the last chunks.
                    dve_first = False
                    dlo = 0 if dve_first else n_act  # first DVE row
                    alo = n_dve if dve_first else 0  # first ACT row
                    for j in range(n_act):
                        nc.scalar.activation(
                            out=prod[:, alo + j, :], in_=sub_tile[:, alo + j, :],
                            func=mybir.ActivationFunctionType.Identity,
                            scale=q_a[:, d0 + alo + j : d0 + alo + j + 1],
                        )
                    if n_dve:
                        q_b = (
                            q_v[:, d0 + dlo : d0 + dlo + n_dve]
                            .unsqueeze(2)
                            .broadcast_to([P, n_dve, NS])
                        )
                        nc.vector.tensor_tensor(
                            out=prod[:, dlo : dlo + n_dve, :],
                            in0=sub_tile[:, dlo : dlo + n_dve, :],
                            in1=q_b, op=mybir.AluOpType.mult,
                        )
                    # in-place halving fold over d: 16 -> 8 -> 4, then
                    # accumulate the [P, 4, NS] remainder (one fewer small
                    # op per slot than folding all the way to 2 rows)
                    k = DH // 2
                    while k >= 4:
                        nc.vector.tensor_tensor(
                            out=prod[:, 0:k, :], in0=prod[:, 0:k, :],
                            in1=prod[:, k : 2 * k, :], op=mybir.AluOpType.add,
                        )
                        k //= 2
                    if h == 0:
                        nc.vector.tensor_copy(corr4[:], prod[:, 0:4, :])
                    else:
                        nc.vector.tensor_tensor(
                            out=corr4[:], in0=corr4[:], in1=prod[:, 0:4, :],
                            op=mybir.AluOpType.add,
                        )

                # fold corr4 to a single [P, NS] row in place
                nc.vector.tensor_tensor(
                    out=corr4[:, 0:2, :], in0=corr4[:, 0:2, :],
                    in1=corr4[:, 2:4, :], op=mybir.AluOpType.add,
                )
                nc.vector.tensor_tensor(
                    out=corr4[:, 0, :], in0=corr4[:, 0, :], in1=corr4[:, 1, :],
                    op=mybir.AluOpType.add,
                )
                _argmax_onehot(nc, c_pool, iota_v, corr4[:, 0, :], out, b0)

    nc.compile()
    return nc


def _build_v6():
    """v6: three-way engine split so every engine stays under the DMA floor.

    Layout [b, d, s] (host-transposed).  Per slot of DH=16 d-rows:
      - N_C rows: ACT multiplies (per-partition scale q) -> PE accumulates
        each row into a PSUM block via exact fp32 identity matmul
        (out[p,s] += I @ prod[p,s]; fp32 matmul is the exact 2-pass mode).
      - N_B rows: ACT multiplies -> DVE folds into a persistent [P,N_B,NS]
        accumulator (keeps ACT/PE/DVE balanced).
      - remaining rows: DVE scalar_tensor_tensor accumulate chains
        (acc = sub*q + acc, 1 op/elem, alternating acc0/acc1 to hide RAW).
    Tail per block: merge acc0+acc1+acc2+psum, then exact argmax one-hot.
    """
    from concourse.masks import make_identity

    nc = bacc.Bacc("TRN2", target_bir_lowering=False, debug=False)
    fq = nc.declare_dram_parameter("feat_query", [BZL, D], F32, isOutput=False)
    fs = nc.declare_dram_parameter("feat_sub", [BZL, D, NS], F32, isOutput=False)
    iota = nc.declare_dram_parameter("iota", [P, NS], F32, isOutput=False)
    out = nc.declare_dram_parameter("out", [BZL, NS], F32, isOutput=True)

    n_c = N_C  # ACT-mult -> PE-accumulate rows per slot
    n_b = N_B  # ACT-mult -> DVE-fold rows per slot
    n_a = DH - n_c - n_b  # DVE STT-chain rows per slot

    with tile.TileContext(nc) as tc:
        with (
            tc.tile_pool(name="sub", bufs=4) as sub_pool,
            tc.tile_pool(name="pc", bufs=4) as pc_pool,
            tc.tile_pool(name="pb", bufs=2) as pb_pool,
            tc.tile_pool(name="qp", bufs=NBLK) as q_pool,
            tc.tile_pool(name="cp", bufs=NBLK) as c_pool,
            tc.tile_pool(name="const", bufs=1) as const_pool,
            tc.psum_pool(name="ps", bufs=2) as psum_pool,
        ):
            iota_v = const_pool.tile([P, NS], F32)
            nc.scalar.dma_start(out=iota_v[:], in_=iota[:, :])
            ident = const_pool.tile([P, P], F32)
            make_identity(nc, ident[:])

            for blk in range(NBLK):
                b0 = blk * P
                q_v = q_pool.tile([P, D], F32)
                nc.scalar.dma_start(out=q_v[:], in_=fq[b0 : b0 + P, :])
                q_a = q_pool.tile([P, D], F32)
                nc.scalar.activation(
                    out=q_a[:], in_=q_v[:],
                    func=mybir.ActivationFunctionType.Identity,
                )
                acc0 = c_pool.tile([P, NS], F32)
                acc1 = c_pool.tile([P, NS], F32)
                acc2 = c_pool.tile([P, n_b, NS], F32)
                # full 2KB zero region per block so a start=True matmul on the
                # next block can't zero this block's still-unread psum
                psum_t = psum_pool.tile([P, 512], F32)
                psum = psum_t[:, 0:NS]

                for h in range(NH):
                    d0 = h * DH
                    sub_tile = sub_pool.tile([P, DH, NS], F32)
                    nsplit = 4 if (blk == 0 and h <= 1) else DMA_SPLIT
                    dstep = DH // nsplit
                    for c in range(nsplit):
                        nc.sync.dma_start(
                            out=sub_tile[:, c * dstep : (c + 1) * dstep, :],
                            in_=fs[b0 : b0 + P, d0 + c * dstep : d0 + (c + 1) * dstep, :],
                        )
                    # ACT rows lead (they arrive in the first DMA chunk)
                    prod_c = pc_pool.tile([P, n_c, NS], F32)
                    for j in range(n_c):
                        nc.scalar.activation(
                            out=prod_c[:, j, :], in_=sub_tile[:, j, :],
                            func=mybir.ActivationFunctionType.Identity,
                            scale=q_a[:, d0 + j : d0 + j + 1],
                        )
                    prod_b = pb_pool.tile([P, n_b, NS], F32)
                    for j in range(n_b):
                        dd = n_c + j
                        nc.scalar.activation(
                            out=prod_b[:, j, :], in_=sub_tile[:, dd, :],
                            func=mybir.ActivationFunctionType.Identity,
                            scale=q_a[:, d0 + dd : d0 + dd + 1],
                        )
                    # PE: exact fp32 identity-matmul accumulation of prod_c
                    for j in range(n_c):
                        nc.tensor.matmul(
                            psum,
                            ident[:],
                            prod_c[:, j, :],
                            start=(h == 0 and j == 0),
                            stop=(h == NH - 1 and j == n_c - 1),
                        )
                    # DVE: fold prod_b into acc2
                    if h == 0:
                        nc.vector.tensor_copy(acc2[:], prod_b[:])
                    else:
                        nc.vector.tensor_tensor(
                            out=acc2[:], in0=acc2[:], in1=prod_b[:],
                            op=mybir.AluOpType.add,
                        )
                    # DVE: STT accumulate chains on the trailing rows
                    for k in range(n_a):
                        dd = n_c + n_b + k
                        d = d0 + dd
                        acc = acc0 if (k % 2 == 0) else acc1
                        if h == 0 and k < 2:
                            nc.vector.tensor_scalar(
                                out=acc[:], in0=sub_tile[:, dd, :],
                                scalar1=q_v[:, d : d + 1], scalar2=None,
                                op0=mybir.AluOpType.mult,
                            )
                        else:
                            nc.vector.scalar_tensor_tensor(
                                out=acc[:], in0=sub_tile[:, dd, :],
                                scalar=q_v[:, d : d + 1], in1=acc[:],
                                op0=mybir.AluOpType.mult, op1=mybir.AluOpType.add,
                            )

                # tail: corr = acc0 + acc1 + sum(acc2 rows) + psum
                for j in range(1, n_b):
                    nc.vector.tensor_tensor(
                        out=acc2[:, 0, :], in0=acc2[:, 0, :], in1=acc2[:, j, :],
                        op=mybir.AluOpType.add,
                    )
                nc.vector.tensor_tensor(
                    out=acc0[:], in0=acc0[:], in1=acc1[:], op=mybir.AluOpType.add
                )
                nc.vector.tensor_tensor(
                    out=acc0[:], in0=acc0[:], in1=acc2[:, 0, :], op=mybir.AluOpType.add
                )
                corr = c_pool.tile([P, NS], F32)
                nc.vector.tensor_tensor(
                    out=corr[:], in0=acc0[:], in1=psum, op=mybir.AluOpType.add
                )
                _argmax_onehot(nc, c_pool, iota_v, corr, out, b0)

    nc.compile()
    return nc


SC = 64  # v1 s-chunk


def _build_v1():
    nc = bacc.Bacc("TRN2", target_bir_lowering=False, debug=False)
    fq = nc.declare_dram_parameter("feat_query", [BZL, D], F32, isOutput=False)
    fs = nc.declare_dram_parameter("feat_sub", [BZL, NS, D], F32, isOutput=False)
    iota = nc.declare_dram_parameter("iota", [P, NS], F32, isOutput=False)
    out = nc.declare_dram_parameter("out", [BZL, NS], F32, isOutput=True)

    with tile.TileContext(nc) as tc:
        with (
            tc.tile_pool(name="sub", bufs=3) as sub_pool,
            tc.tile_pool(name="prod", bufs=2) as prod_pool,
            tc.tile_pool(name="qp", bufs=NBLK) as q_pool,
            tc.tile_pool(name="cp", bufs=NBLK) as c_pool,
            tc.tile_pool(name="const", bufs=1) as const_pool,
        ):
            iota_d = const_pool.tile([P, NS], F32)
            nc.scalar.dma_start(out=iota_d[:], in_=iota[:, :])
            iota_v = const_pool.tile([P, NS], F32)
            nc.vector.tensor_copy(iota_v[:], iota_d[:])

            for blk in range(NBLK):
                b0 = blk * P
                q_d = q_pool.tile([P, D], F32)
                nc.scalar.dma_start(out=q_d[:], in_=fq[b0 : b0 + P, :])
                q_v = q_pool.tile([P, D], F32)
                nc.vector.tensor_copy(q_v[:], q_d[:])
                corr = c_pool.tile([P, NS], F32)

                for ci in range(NS // SC):
                    sub_tile = sub_pool.tile([P, SC, D], F32)
                    nc.sync.dma_start(
                        out=sub_tile[:],
                        in_=fs[b0 : b0 + P, ci * SC : (ci + 1) * SC, :],
                    )
                    prod = prod_pool.tile([P, SC, D], F32)
                    q_b = q_v[:, :].unsqueeze(1).broadcast_to([P, SC, D])
                    nc.vector.tensor_tensor(
                        out=prod[:], in0=sub_tile[:], in1=q_b, op=mybir.AluOpType.mult
                    )
                    nc.vector.reduce_sum(
                        out=corr[:, ci * SC : (ci + 1) * SC],
                        in_=prod[:],
                        axis=mybir.AxisListType.X,
                    )

                _argmax_onehot(nc, c_pool, iota_v, corr, out, b0)

    nc.compile()
    return nc


_CACHE = {}


def _get_nc():
    key = f"{VARIANT}-{DH}-{D_ACT}-{N_C}-{N_B}-{DMA_SPLIT}-{ACC_DMA}"
    if key not in _CACHE:
        builders = {"v1": _build_v1, "v2": _build_v2, "v3": _build_v3, "v4": _build_v4, "v6": _build_v6}
        _CACHE[key] = builders[VARIANT]()
    return _CACHE[key]


def _in_maps(feat_query, feat_sub):
    feat_query = np.ascontiguousarray(np.asarray(feat_query), dtype=np.float32)
    feat_sub = np.asarray(feat_sub)
    assert feat_query.shape == (BZ, D), feat_query.shape
    assert feat_sub.shape == (BZ, NS, D), feat_sub.shape
    if VARIANT == "v2":
        # host-side reorder: [BZ, NS, D] -> [BZ, NH, NS, DH] (d-slices contiguous)
        feat_sub = np.ascontiguousarray(
            feat_sub.reshape(BZ, NS, NH, DH).transpose(0, 2, 1, 3), dtype=np.float32
        )
    elif VARIANT in ("v3", "v4", "v6"):
        # host-side transpose: [BZ, NS, D] -> [BZ, D, NS]
        feat_sub = np.ascontiguousarray(
            feat_sub.transpose(0, 2, 1), dtype=np.float32
        )
    else:
        feat_sub = np.ascontiguousarray(feat_sub, dtype=np.float32)
    iota_np = np.tile(np.arange(NS, dtype=np.float32) - 1024.0, (P, 1))
    maps = []
    for i in range(N_CORES):
        sl = slice(i * BZL, (i + 1) * BZL)
        maps.append(
            {"feat_query": feat_query[sl], "feat_sub": feat_sub[sl], "iota": iota_np}
        )
    return maps


def _assemble(results):
    outs = [results[i]["out"] for i in range(N_CORES)]
    return np.concatenate(outs, axis=0).reshape(BZ, NS, 1).astype(np.float32)


def run(feat_query, feat_sub, trace=False):
    """Run on 8 NeuronCores; returns (output, BassKernelResults)."""
    nc = _get_nc()
    res = run_bass_kernel_spmd(
        nc, _in_maps(feat_query, feat_sub), list(range(N_CORES)), trace=trace
    )
    return _assemble(res.results), res


def kernel(feat_query, feat_sub):
    out, _ = run(feat_query, feat_sub, trace=False)
    return out



# revision 9
# speedup vs baseline: 1.1205x; 1.1088x over previous
"""Trainium2 Bass kernel for nn_AttentionMax (batched dot-product argmax one-hot).

corr[b, s] = <feat_query[b], feat_sub[b, s]>   (bz=4096, n_support=256, d=128)
out[b, s, 0] = one_hot(argmax_s corr[b])

Sharding: pure data parallel over the batch dim across 8 NeuronCores
(512 batches per core = blocks of 128; partition dim = batch).

Active strategy (VARIANT="v4"): feat_sub is transposed on the host to
[b, d, s] so each [P, DH, NS] slot DMAs contiguously.  Per slot, the
multiplies prod[d] = sub[d] * q[d] are split between VectorE (one big
tensor_tensor with q 0-stride-broadcast along s) and ScalarE (per-d
activation Identity with per-partition scale) — both engines run at
full tilt in parallel.  The reduction over d is a log2 in-place fold of
tensor_tensor adds (all ops innermost-contiguous; TENSOR_REDUCE over a
strided view measured 1.6x slower, and fp32 PE matmul is a non-starter
since it lowers to 2 LDWEIGHTS+MATMUL passes).  The first-argmax
one-hot is exact (ties resolve to the lowest index, matching
jnp.argmax) via reduce_max -> (corr==max)*(iota-1024) -> reduce_min ->
(iota-1024==min), computed entirely on VectorE.

Measured on hardware: ~230.7 us end-to-end per core, exact output
(memory roofline for the 512 MiB feat_sub stream is ~180 us; VectorE +
ScalarE combined fp32 throughput ~1.38 elem/ns is the binding
constraint; DVE 199 us / ACT 194 us busy, both >82%, ~25 us residual
startup ramp + drain-barrier latency).  Config: DH=16 slots, D_ACT
11/12 alternating, DMA split 2 (4 for the first two slots), ScalarE
takes the leading d's of each slot so it starts on the first chunk,
fold stops at a [P, 4, NS] running accumulator, and compute reads the
q/iota DMA tiles directly (Bacc legalizes multi-wait instructions, so
no staging copies are needed).
"""

import sys

if "/opt/trn_rl_repo" not in sys.path:
    sys.path.insert(0, "/opt/trn_rl_repo")

import numpy as np

import concourse.bass as bass
import concourse.mybir as mybir
from concourse import bacc, tile
from concourse.bass_utils import run_bass_kernel_spmd

N_CORES = 8
BZ = 4096
BZL = BZ // N_CORES  # 512 batches per core
NS = 256  # n_support
D = 128
P = 128  # batches per block (partition dim)
NBLK = BZL // P  # 4

# v2 layout params
DH = 16  # d-slice width per DMA slot
NH = D // DH  # slots per block
D_ACT = 11  # per-slot count of d's offloaded to ScalarE (v3/v4)
N_C = 7  # v6: ACT->PE rows per slot
N_B = 0  # v6: ACT->DVE-fold rows per slot
DMA_SPLIT = 2  # per-slot DMA split (v4): finer chunks -> earlier compute start
ACC_DMA = False  # SWDGE accum DMA for corr2: measured 27 us WORSE (serial RMW latency)

VARIANT = "v6"

F32 = mybir.dt.float32


def _argmax_onehot(nc, c_pool, iota_v, acc, out, b0):
    """Exact first-argmax one-hot from acc [P, NS] -> DMA to out[b0:b0+P]."""
    rmax = c_pool.tile([P, 1], F32)
    nc.vector.reduce_max(out=rmax[:], in_=acc[:], axis=mybir.AxisListType.X)
    masked = c_pool.tile([P, NS], F32)
    nc.vector.scalar_tensor_tensor(
        out=masked[:], in0=acc[:], scalar=rmax[:], in1=iota_v[:],
        op0=mybir.AluOpType.is_equal, op1=mybir.AluOpType.mult,
    )
    rmin = c_pool.tile([P, 1], F32)
    nc.vector.tensor_reduce(
        out=rmin[:], in_=masked[:], axis=mybir.AxisListType.X,
        op=mybir.AluOpType.min,
    )
    onehot = c_pool.tile([P, NS], F32)
    nc.vector.tensor_scalar(
        out=onehot[:], in0=iota_v[:], scalar1=rmin[:], scalar2=None,
        op0=mybir.AluOpType.is_equal,
    )
    nc.scalar.dma_start(out=out[b0 : b0 + P, :], in_=onehot[:])


def _build_v2():
    nc = bacc.Bacc("TRN2", target_bir_lowering=False, debug=False)
    fq = nc.declare_dram_parameter("feat_query", [BZL, D], F32, isOutput=False)
    fs = nc.declare_dram_parameter("feat_sub", [BZL, NH, NS, DH], F32, isOutput=False)
    iota = nc.declare_dram_parameter("iota", [P, NS], F32, isOutput=False)
    out = nc.declare_dram_parameter("out", [BZL, NS], F32, isOutput=True)

    n_act = D_ACT  # per-slot count of ACT-offloaded d's
    with tile.TileContext(nc) as tc:
        with (
            tc.tile_pool(name="sub", bufs=3) as sub_pool,
            tc.tile_pool(name="qp", bufs=NBLK) as q_pool,
            tc.tile_pool(name="cp", bufs=NBLK) as c_pool,
            tc.tile_pool(name="pa", bufs=2) as pa_pool,
            tc.tile_pool(name="const", bufs=1) as const_pool,
        ):
            iota_d = const_pool.tile([P, NS], F32)
            nc.scalar.dma_start(out=iota_d[:], in_=iota[:, :])
            iota_v = const_pool.tile([P, NS], F32)
            nc.vector.tensor_copy(iota_v[:], iota_d[:])

            for blk in range(NBLK):
                b0 = blk * P
                q_d = q_pool.tile([P, D], F32)
                nc.scalar.dma_start(out=q_d[:], in_=fq[b0 : b0 + P, :])
                q_v = q_pool.tile([P, D], F32)
                nc.vector.tensor_copy(q_v[:], q_d[:])
                # ScalarE also needs q as its scale operand; give it its own
                # copy so ACT ops don't add cross-engine waits against DVE.
                if n_act:
                    q_a = q_pool.tile([P, D], F32)
                    nc.scalar.activation(
                        out=q_a[:], in_=q_d[:],
                        func=mybir.ActivationFunctionType.Identity,
                    )
                    prod_a = pa_pool.tile([P, NS, NH * n_act], F32)
                acc = c_pool.tile([P, NS], F32)

                for h in range(NH):
                    sub_tile = sub_pool.tile([P, NS, DH], F32)
                    nc.sync.dma_start(out=sub_tile[:], in_=fs[b0 : b0 + P, h, :, :])
                    n_dve = DH - n_act
                    for dd in range(n_dve):
                        d = h * DH + dd
                        if d == 0:
                            nc.vector.tensor_scalar(
                                out=acc[:], in0=sub_tile[:, :, 0],
                                scalar1=q_v[:, 0:1], scalar2=None,
                                op0=mybir.AluOpType.mult,
                            )
                        else:
                            nc.vector.scalar_tensor_tensor(
                                out=acc[:], in0=sub_tile[:, :, dd],
                                scalar=q_v[:, d : d + 1], in1=acc[:],
                                op0=mybir.AluOpType.mult, op1=mybir.AluOpType.add,
                            )
                    for j in range(n_act):
                        dd = n_dve + j
                        d = h * DH + dd
                        nc.scalar.activation(
                            out=prod_a[:, :, h * n_act + j], in_=sub_tile[:, :, dd],
                            func=mybir.ActivationFunctionType.Identity,
                            scale=q_a[:, d : d + 1],
                        )

                if n_act:
                    psum_a = c_pool.tile([P, NS], F32)
                    nc.vector.reduce_sum(
                        out=psum_a[:], in_=prod_a[:], axis=mybir.AxisListType.X
                    )
                    nc.vector.tensor_tensor(
                        out=acc[:], in0=acc[:], in1=psum_a[:], op=mybir.AluOpType.add
                    )

                _argmax_onehot(nc, c_pool, iota_v, acc, out, b0)

    nc.compile()
    return nc


def _build_v3():
    """Layout [b, d, s]: slots [P, DH, NS] (contiguous per partition).

    Per slot of DH d-values: DVE multiplies the first DH-D_ACT d's in one
    big tensor_tensor (q broadcast along s), ScalarE multiplies the other
    D_ACT d's (contiguous activations with per-partition scale) into the
    same prod tile.  DVE then reduce_sums the slot over d via an s-major
    strided view and accumulates partial correlations.
    """
    nc = bacc.Bacc("TRN2", target_bir_lowering=False, debug=False)
    fq = nc.declare_dram_parameter("feat_query", [BZL, D], F32, isOutput=False)
    fs = nc.declare_dram_parameter("feat_sub", [BZL, D, NS], F32, isOutput=False)
    iota = nc.declare_dram_parameter("iota", [P, NS], F32, isOutput=False)
    out = nc.declare_dram_parameter("out", [BZL, NS], F32, isOutput=True)

    n_act = D_ACT
    n_dve = DH - n_act
    with tile.TileContext(nc) as tc:
        with (
            tc.tile_pool(name="sub", bufs=3) as sub_pool,
            tc.tile_pool(name="prod", bufs=2) as prod_pool,
            tc.tile_pool(name="qp", bufs=NBLK) as q_pool,
            tc.tile_pool(name="cp", bufs=NBLK) as c_pool,
            tc.tile_pool(name="const", bufs=1) as const_pool,
        ):
            iota_d = const_pool.tile([P, NS], F32)
            nc.scalar.dma_start(out=iota_d[:], in_=iota[:, :])
            iota_v = const_pool.tile([P, NS], F32)
            nc.vector.tensor_copy(iota_v[:], iota_d[:])

            for blk in range(NBLK):
                b0 = blk * P
                q_d = q_pool.tile([P, D], F32)
                nc.scalar.dma_start(out=q_d[:], in_=fq[b0 : b0 + P, :])
                q_v = q_pool.tile([P, D], F32)
                nc.vector.tensor_copy(q_v[:], q_d[:])
                q_a = q_pool.tile([P, D], F32)
                nc.scalar.activation(
                    out=q_a[:], in_=q_d[:],
                    func=mybir.ActivationFunctionType.Identity,
                )
                corr = c_pool.tile([P, NS], F32)

                for h in range(NH):
                    d0 = h * DH
                    sub_tile = sub_pool.tile([P, DH, NS], F32)
                    nc.sync.dma_start(out=sub_tile[:], in_=fs[b0 : b0 + P, d0 : d0 + DH, :])
                    prod = prod_pool.tile([P, DH, NS], F32)
                    if n_dve:
                        q_b = (
                            q_v[:, d0 : d0 + n_dve]
                            .unsqueeze(2)
                            .broadcast_to([P, n_dve, NS])
                        )
                        nc.vector.tensor_tensor(
                            out=prod[:, 0:n_dve, :], in0=sub_tile[:, 0:n_dve, :],
                            in1=q_b, op=mybir.AluOpType.mult,
                        )
                    for j in range(n_act):
                        dd = n_dve + j
                        nc.scalar.activation(
                            out=prod[:, dd, :], in_=sub_tile[:, dd, :],
                            func=mybir.ActivationFunctionType.Identity,
                            scale=q_a[:, d0 + dd : d0 + dd + 1],
                        )
                    # reduce over d via s-major strided view
                    psum_h = c_pool.tile([P, NS], F32)
                    nc.vector.reduce_sum(
                        out=psum_h[:],
                        in_=prod[:].rearrange("p d s -> p s d"),
                        axis=mybir.AxisListType.X,
                    )
                    if h == 0:
                        first = psum_h
                    else:
                        nc.vector.tensor_tensor(
                            out=corr[:] if h == NH - 1 else first[:],
                            in0=first[:], in1=psum_h[:], op=mybir.AluOpType.add,
                        )

                _argmax_onehot(nc, c_pool, iota_v, corr, out, b0)

    nc.compile()
    return nc


def _build_v4():
    """Layout [b, d, s] with TT-add fold reduction (all ops inner-contiguous).

    Per slot of DH=32 d-values: DVE multiplies the first DH-D_ACT d's in one
    tensor_tensor (q broadcast along s), ScalarE multiplies the other D_ACT
    d's (contiguous in/out, per-partition scale).  The d-reduction is a
    log2 fold of in-place tensor_tensor adds on [P, k, NS] slices -- every
    op reads/writes s-contiguous memory (no strided TENSOR_REDUCE).
    """
    nc = bacc.Bacc("TRN2", target_bir_lowering=False, debug=False)
    fq = nc.declare_dram_parameter("feat_query", [BZL, D], F32, isOutput=False)
    fs = nc.declare_dram_parameter("feat_sub", [BZL, D, NS], F32, isOutput=False)
    iota = nc.declare_dram_parameter("iota", [P, NS], F32, isOutput=False)
    out = nc.declare_dram_parameter("out", [BZL, NS], F32, isOutput=True)

    with tile.TileContext(nc) as tc:
        with (
            tc.tile_pool(name="sub", bufs=4) as sub_pool,
            tc.tile_pool(name="prod", bufs=5) as prod_pool,
            tc.tile_pool(name="qp", bufs=NBLK) as q_pool,
            tc.tile_pool(name="cp", bufs=NBLK) as c_pool,
            tc.tile_pool(name="const", bufs=1) as const_pool,
        ):
            iota_v = const_pool.tile([P, NS], F32)
            nc.scalar.dma_start(out=iota_v[:], in_=iota[:, :])

            for blk in range(NBLK):
                b0 = blk * P
                q_v = q_pool.tile([P, D], F32)
                nc.scalar.dma_start(out=q_v[:], in_=fq[b0 : b0 + P, :])
                q_a = q_pool.tile([P, D], F32)
                nc.scalar.activation(
                    out=q_a[:], in_=q_v[:],
                    func=mybir.ActivationFunctionType.Identity,
                )
                corr4 = c_pool.tile([P, 4, NS], F32)

                for h in range(NH):
                    d0 = h * DH
                    # alternate ScalarE share to balance engine busy-time;
                    # ACT-heavier at the tail of the last block so the final
                    # DVE-only fold+argmax stretch is shorter
                    n_act = D_ACT + ((blk * NH + h) % 2)
                    if blk == NBLK - 1 and h >= NH - 2:
                        n_act += 2
                    n_dve = DH - n_act
                    sub_tile = sub_pool.tile([P, DH, NS], F32)
                    # finer chunks for the first two slots: compute starts
                    # during the DMA ramp instead of after the first 1 MB
                    nsplit = 4 if (blk == 0 and h <= 1) else DMA_SPLIT
                    dstep = DH // nsplit
                    for c in range(nsplit):
                        nc.sync.dma_start(
                            out=sub_tile[:, c * dstep : (c + 1) * dstep, :],
                            in_=fs[b0 : b0 + P, d0 + c * dstep : d0 + (c + 1) * dstep, :],
                        )
                    prod = prod_pool.tile([P, DH, NS], F32)
                    # ScalarE takes the LEADING d's: they arrive in the first
                    # DMA chunk, so ACT starts as early as possible; VectorE
                    # multiplies the trailing d's in one big tensor_tensor.
                    # Exception: the very first slot flips the assignment --
                    # at kernel start the DMA ramp is slow and VectorE (the
                    # critical engine) would otherwise idle ~10 us waiting
                    # for the last chunks.
                    dve_first = False
                    dlo = 0 if dve_first else n_act  # first DVE row
                    alo = n_dve if dve_first else 0  # first ACT row
                    for j in range(n_act):
                        nc.scalar.activation(
                            out=prod[:, alo + j, :], in_=sub_tile[:, alo + j, :],
                            func=mybir.ActivationFunctionType.Identity,
                            scale=q_a[:, d0 + alo + j : d0 + alo + j + 1],
                        )
                    if n_dve:
                        q_b = (
                            q_v[:, d0 + dlo : d0 + dlo + n_dve]
                            .unsqueeze(2)
                            .broadcast_to([P, n_dve, NS])
                        )
                        nc.vector.tensor_tensor(
                            out=prod[:, dlo : dlo + n_dve, :],
                            in0=sub_tile[:, dlo : dlo + n_dve, :],
                            in1=q_b, op=mybir.AluOpType.mult,
                        )
                    # in-place halving fold over d: 16 -> 8 -> 4, then
                    # accumulate the [P, 4, NS] remainder (one fewer small
                    # op per slot than folding all the way to 2 rows)
                    k = DH // 2
                    while k >= 4:
                        nc.vector.tensor_tensor(
                            out=prod[:, 0:k, :], in0=prod[:, 0:k, :],
                            in1=prod[:, k : 2 * k, :], op=mybir.AluOpType.add,
                        )
                        k //= 2
                    if h == 0:
                        nc.vector.tensor_copy(corr4[:], prod[:, 0:4, :])
                    else:
                        nc.vector.tensor_tensor(
                            out=corr4[:], in0=corr4[:], in1=prod[:, 0:4, :],
                            op=mybir.AluOpType.add,
                        )

                # fold corr4 to a single [P, NS] row in place
                nc.vector.tensor_tensor(
                    out=corr4[:, 0:2, :], in0=corr4[:, 0:2, :],
                    in1=corr4[:, 2:4, :], op=mybir.AluOpType.add,
                )
                nc.vector.tensor_tensor(
                    out=corr4[:, 0, :], in0=corr4[:, 0, :], in1=corr4[:, 1, :],
                    op=mybir.AluOpType.add,
                )
                _argmax_onehot(nc, c_pool, iota_v, corr4[:, 0, :], out, b0)

    nc.compile()
    return nc


def _build_v6():
    """v6: three-way engine split so every engine stays under the DMA floor.

    Layout [b, d, s] (host-transposed).  Per slot of DH=16 d-rows:
      - N_C rows: ACT multiplies (per-partition scale q) -> PE accumulates
        each row into a PSUM block via exact fp32 identity matmul
        (out[p,s] += I @ prod[p,s]; fp32 matmul is the exact 2-pass mode).
      - N_B rows: ACT multiplies -> DVE folds into a persistent [P,N_B,NS]
        accumulator (keeps ACT/PE/DVE balanced).
      - remaining rows: DVE scalar_tensor_tensor accumulate chains
        (acc = sub*q + acc, 1 op/elem, alternating acc0/acc1 to hide RAW).
    Tail per block: merge acc0+acc1+acc2+psum, then exact argmax one-hot.
    """
    from concourse.masks import make_identity

    nc = bacc.Bacc("TRN2", target_bir_lowering=False, debug=False)
    fq = nc.declare_dram_parameter("feat_query", [BZL, D], F32, isOutput=False)
    fs = nc.declare_dram_parameter("feat_sub", [BZL, D, NS], F32, isOutput=False)
    iota = nc.declare_dram_parameter("iota", [P, NS], F32, isOutput=False)
    out = nc.declare_dram_parameter("out", [BZL, NS], F32, isOutput=True)

    n_c = N_C  # ACT-mult -> PE-accumulate rows per slot
    n_b = N_B  # ACT-mult -> DVE-fold rows per slot
    n_a = DH - n_c - n_b  # DVE STT-chain rows per slot

    with tile.TileContext(nc) as tc:
        with (
            tc.tile_pool(name="sub", bufs=6) as sub_pool,
            tc.tile_pool(name="pc", bufs=4) as pc_pool,
            tc.tile_pool(name="pb", bufs=2) as pb_pool,
            tc.tile_pool(name="qp", bufs=NBLK) as q_pool,
            tc.tile_pool(name="cp", bufs=NBLK) as c_pool,
            tc.tile_pool(name="const", bufs=1) as const_pool,
            tc.psum_pool(name="ps", bufs=2) as psum_pool,
        ):
            iota_v = const_pool.tile([P, NS], F32)
            nc.scalar.dma_start(out=iota_v[:], in_=iota[:, :])
            ident = const_pool.tile([P, P], F32)
            make_identity(nc, ident[:])

            for blk in range(NBLK):
                b0 = blk * P
                q_v = q_pool.tile([P, D], F32)
                nc.scalar.dma_start(out=q_v[:], in_=fq[b0 : b0 + P, :])
                q_a = q_pool.tile([P, D], F32)
                nc.scalar.activation(
                    out=q_a[:], in_=q_v[:],
                    func=mybir.ActivationFunctionType.Identity,
                )
                acc0 = c_pool.tile([P, NS], F32)
                acc1 = c_pool.tile([P, NS], F32)
                acc2 = c_pool.tile([P, n_b, NS], F32) if n_b else None
                # full 2KB zero region per block so a start=True matmul on the
                # next block can't zero this block's still-unread psum
                psum_t = psum_pool.tile([P, 512], F32)
                psum = psum_t[:, 0:NS]

                for h in range(NH):
                    d0 = h * DH
                    sub_tile = sub_pool.tile([P, DH, NS], F32)
                    nsplit = 4 if (blk == 0 and h <= 1) else DMA_SPLIT
                    dstep = DH // nsplit
                    for c in range(nsplit):
                        nc.sync.dma_start(
                            out=sub_tile[:, c * dstep : (c + 1) * dstep, :],
                            in_=fs[b0 : b0 + P, d0 + c * dstep : d0 + (c + 1) * dstep, :],
                        )
                    # ACT rows lead (they arrive in the first DMA chunk)
                    prod_c = pc_pool.tile([P, n_c, NS], F32)
                    for j in range(n_c):
                        nc.scalar.activation(
                            out=prod_c[:, j, :], in_=sub_tile[:, j, :],
                            func=mybir.ActivationFunctionType.Identity,
                            scale=q_a[:, d0 + j : d0 + j + 1],
                        )
                    if n_b:
                        prod_b = pb_pool.tile([P, n_b, NS], F32)
                        for j in range(n_b):
                            dd = n_c + j
                            nc.scalar.activation(
                                out=prod_b[:, j, :], in_=sub_tile[:, dd, :],
                                func=mybir.ActivationFunctionType.Identity,
                                scale=q_a[:, d0 + dd : d0 + dd + 1],
                            )
                    # PE: exact fp32 identity-matmul accumulation of prod_c
                    for j in range(n_c):
                        nc.tensor.matmul(
                            psum,
                            ident[:],
                            prod_c[:, j, :],
                            start=(h == 0 and j == 0),
                            stop=(h == NH - 1 and j == n_c - 1),
                        )
                    # DVE: fold prod_b into acc2
                    if n_b:
                        if h == 0:
                            nc.vector.tensor_copy(acc2[:], prod_b[:])
                        else:
                            nc.vector.tensor_tensor(
                                out=acc2[:], in0=acc2[:], in1=prod_b[:],
                                op=mybir.AluOpType.add,
                            )
                    # DVE: STT accumulate chains on the trailing rows
                    for k in range(n_a):
                        dd = n_c + n_b + k
                        d = d0 + dd
                        acc = acc0 if (k % 2 == 0) else acc1
                        if h == 0 and k < 2:
                            nc.vector.tensor_scalar(
                                out=acc[:], in0=sub_tile[:, dd, :],
                                scalar1=q_v[:, d : d + 1], scalar2=None,
                                op0=mybir.AluOpType.mult,
                            )
                        else:
                            nc.vector.scalar_tensor_tensor(
                                out=acc[:], in0=sub_tile[:, dd, :],
                                scalar=q_v[:, d : d + 1], in1=acc[:],
                                op0=mybir.AluOpType.mult, op1=mybir.AluOpType.add,
                            )

                # tail: corr = acc0 + acc1 + sum(acc2 rows) + psum
                for j in range(1, n_b):
                    nc.vector.tensor_tensor(
                        out=acc2[:, 0, :], in0=acc2[:, 0, :], in1=acc2[:, j, :],
                        op=mybir.AluOpType.add,
                    )
                nc.vector.tensor_tensor(
                    out=acc0[:], in0=acc0[:], in1=acc1[:], op=mybir.AluOpType.add
                )
                if n_b:
                    nc.vector.tensor_tensor(
                        out=acc0[:], in0=acc0[:], in1=acc2[:, 0, :],
                        op=mybir.AluOpType.add,
                    )
                corr = c_pool.tile([P, NS], F32)
                nc.vector.tensor_tensor(
                    out=corr[:], in0=acc0[:], in1=psum, op=mybir.AluOpType.add
                )
                _argmax_onehot(nc, c_pool, iota_v, corr, out, b0)

    nc.compile()
    return nc


SC = 64  # v1 s-chunk


def _build_v1():
    nc = bacc.Bacc("TRN2", target_bir_lowering=False, debug=False)
    fq = nc.declare_dram_parameter("feat_query", [BZL, D], F32, isOutput=False)
    fs = nc.declare_dram_parameter("feat_sub", [BZL, NS, D], F32, isOutput=False)
    iota = nc.declare_dram_parameter("iota", [P, NS], F32, isOutput=False)
    out = nc.declare_dram_parameter("out", [BZL, NS], F32, isOutput=True)

    with tile.TileContext(nc) as tc:
        with (
            tc.tile_pool(name="sub", bufs=3) as sub_pool,
            tc.tile_pool(name="prod", bufs=2) as prod_pool,
            tc.tile_pool(name="qp", bufs=NBLK) as q_pool,
            tc.tile_pool(name="cp", bufs=NBLK) as c_pool,
            tc.tile_pool(name="const", bufs=1) as const_pool,
        ):
            iota_d = const_pool.tile([P, NS], F32)
            nc.scalar.dma_start(out=iota_d[:], in_=iota[:, :])
            iota_v = const_pool.tile([P, NS], F32)
            nc.vector.tensor_copy(iota_v[:], iota_d[:])

            for blk in range(NBLK):
                b0 = blk * P
                q_d = q_pool.tile([P, D], F32)
                nc.scalar.dma_start(out=q_d[:], in_=fq[b0 : b0 + P, :])
                q_v = q_pool.tile([P, D], F32)
                nc.vector.tensor_copy(q_v[:], q_d[:])
                corr = c_pool.tile([P, NS], F32)

                for ci in range(NS // SC):
                    sub_tile = sub_pool.tile([P, SC, D], F32)
                    nc.sync.dma_start(
                        out=sub_tile[:],
                        in_=fs[b0 : b0 + P, ci * SC : (ci + 1) * SC, :],
                    )
                    prod = prod_pool.tile([P, SC, D], F32)
                    q_b = q_v[:, :].unsqueeze(1).broadcast_to([P, SC, D])
                    nc.vector.tensor_tensor(
                        out=prod[:], in0=sub_tile[:], in1=q_b, op=mybir.AluOpType.mult
                    )
                    nc.vector.reduce_sum(
                        out=corr[:, ci * SC : (ci + 1) * SC],
                        in_=prod[:],
                        axis=mybir.AxisListType.X,
                    )

                _argmax_onehot(nc, c_pool, iota_v, corr, out, b0)

    nc.compile()
    return nc


_CACHE = {}


def _get_nc():
    key = f"{VARIANT}-{DH}-{D_ACT}-{N_C}-{N_B}-{DMA_SPLIT}-{ACC_DMA}"
    if key not in _CACHE:
        builders = {"v1": _build_v1, "v2": _build_v2, "v3": _build_v3, "v4": _build_v4, "v6": _build_v6}
        _CACHE[key] = builders[VARIANT]()
    return _CACHE[key]


def _in_maps(feat_query, feat_sub):
    feat_query = np.ascontiguousarray(np.asarray(feat_query), dtype=np.float32)
    feat_sub = np.asarray(feat_sub)
    assert feat_query.shape == (BZ, D), feat_query.shape
    assert feat_sub.shape == (BZ, NS, D), feat_sub.shape
    if VARIANT == "v2":
        # host-side reorder: [BZ, NS, D] -> [BZ, NH, NS, DH] (d-slices contiguous)
        feat_sub = np.ascontiguousarray(
            feat_sub.reshape(BZ, NS, NH, DH).transpose(0, 2, 1, 3), dtype=np.float32
        )
    elif VARIANT in ("v3", "v4", "v6"):
        # host-side transpose: [BZ, NS, D] -> [BZ, D, NS]
        feat_sub = np.ascontiguousarray(
            feat_sub.transpose(0, 2, 1), dtype=np.float32
        )
    else:
        feat_sub = np.ascontiguousarray(feat_sub, dtype=np.float32)
    iota_np = np.tile(np.arange(NS, dtype=np.float32) - 1024.0, (P, 1))
    maps = []
    for i in range(N_CORES):
        sl = slice(i * BZL, (i + 1) * BZL)
        maps.append(
            {"feat_query": feat_query[sl], "feat_sub": feat_sub[sl], "iota": iota_np}
        )
    return maps


def _assemble(results):
    outs = [results[i]["out"] for i in range(N_CORES)]
    return np.concatenate(outs, axis=0).reshape(BZ, NS, 1).astype(np.float32)


def run(feat_query, feat_sub, trace=False):
    """Run on 8 NeuronCores; returns (output, BassKernelResults)."""
    nc = _get_nc()
    res = run_bass_kernel_spmd(
        nc, _in_maps(feat_query, feat_sub), list(range(N_CORES)), trace=trace
    )
    return _assemble(res.results), res


def kernel(feat_query, feat_sub):
    out, _ = run(feat_query, feat_sub, trace=False)
    return out

